# revision 1
# baseline (speedup 1.0000x reference)
"""GAT segment-softmax reduce (nn_GATReduce) for 8 Trainium2 NeuronCores.

Strategy:
  - Host: sort edges by dst (CSR-ization), fold the a1[dst] gather into a
    per-edge score s = a1[dst] + a2, split nodes into 8 contiguous ranges
    (49 blocks of 128 nodes each); every core fully owns its node range so
    no collectives are needed.
  - Softmax without segment-max: inputs are bounded (|s| < ~10) so
    exp(lrelu(s)) is safe in fp32 and softmax is shift-invariant.
  - Device (per core): for each 128-node block, k edge tiles of 128 sorted
    edges, padded so every block has exactly k tiles (pad edges get
    s = -1e9 -> ex = exp(-1e7) = 0, contributing nothing). Per block, DVE
    work is batched into single large ops via broadcast access patterns
    (per-op overhead dominates DVE cost on TRN2):
      * one-hot oh[e,t,n] = (iota[n] == dstl[e,t]) in ONE tensor_tensor
      * vals[e,t,h,d] = ex[e,t,h] * ft[e,t,h,d] in one DVE + one GPSIMD
        tensor_tensor (split across engines for load balance)
      * exp(leaky_relu(s)) batched on ScalarE, written into vals cols
        256:260 so ONE matmul per tile produces numerator and denominator
        together in one PSUM bank (accumulated over the k tiles).
    Epilogue per block: reciprocal of denominator + broadcast multiply.
"""

import math

import numpy as np

import concourse.bacc as bacc
import concourse.mybir as mybir
import concourse.tile as tile
from concourse.bass_utils import run_bass_kernel_spmd

P = 128          # partition count / node block size / edge tile size
H = 4            # heads
D = 64           # feature dim
HD = H * D       # 256
N_CORES = 8

_kernel_cache = {}
LAST_RESULT = None
LAST_NC = None
LAST_IN_MAPS = None

# kernel variant flags (must match between _build and input packing)
OH_BF16 = False
GP_TILES = 4


def _build(nblk: int, k: int, reps: int = 1, bf16_oh: bool = False,
           gp_tiles: int = 0, act_lrelu: bool = False, gp_epi: bool = False,
           f32r: bool = False, dma_split: bool = False, big_bufs: bool = False,
           pair_dma: bool = False, psum_bufs: int = 4):
    """Build the single-core Bass program (SPMD across 8 cores).

    DVE work is batched per node block (not per edge tile) using
    broadcast access patterns, since per-op overhead dominates DVE cost.
    `reps` repeats the whole workload inside one NEFF (for timing).
    `bf16_oh`: feed the is_equal compare bf16 inputs (exact for 0..127).
    `gp_tiles`: offload the vals multiply for the last `gp_tiles` edge
    tiles of each block to GPSIMD.
    """
    assert not bf16_oh, "bf16_oh retired (measured slower); cmp dtype is f32"
    gp_tiles = max(0, min(gp_tiles, k - 1))
    nc = bacc.Bacc("TRN2", target_bir_lowering=False, debug=False)
    f32 = mybir.dt.float32
    mm_dt = mybir.dt.float32r if f32r else f32
    cmp_dt = f32
    # meta packs s (k*H cols) and dstl (k cols) into one per-block DMA
    s_i = nc.dram_tensor("s_i", [nblk, P, k * H + k], f32, kind="ExternalInput")
    # ft pre-swizzled on host to [block, partition, tile, d] so each
    # partition's per-block data is one contiguous k*HD*4 B run in DRAM
    ft_i = nc.dram_tensor("ft_i", [nblk, P, k, HD], f32, kind="ExternalInput")
    iota_i = nc.dram_tensor("iota_i", [P, P], cmp_dt, kind="ExternalInput")
    out_o = nc.dram_tensor("out_o", [nblk * P, HD], f32, kind="ExternalOutput")

    ft_v = ft_i
    if pair_dma:
        npair = nblk // 2
        # paired views: two consecutive blocks per DMA (bigger transfers)
        ft_v2 = ft_i[: npair * 2].rearrange(
            "(b two) p t d -> b p two t d", two=2
        )
        s_v2 = s_i[: npair * 2].rearrange("(b two) p m -> b p two m", two=2)

    with tile.TileContext(nc) as tc:
        small_dma = nc.scalar if dma_split else nc.sync
        with (
            tc.tile_pool(name="const", bufs=1) as cp,
            tc.tile_pool(name="ftp2", bufs=3) as ftp2,
            tc.tile_pool(name="ftp", bufs=2 if pair_dma else (8 if big_bufs else 6)) as ftp,
            tc.tile_pool(name="meta", bufs=8 if big_bufs else 6) as mp,
            tc.tile_pool(name="work", bufs=4) as wp,
            tc.tile_pool(name="ohp", bufs=5 if big_bufs else 4) as ohp,
            tc.tile_pool(name="valp", bufs=5 if big_bufs else 4) as vp,
            tc.tile_pool(name="outp", bufs=3) as op_,
            tc.tile_pool(name="psum", bufs=psum_bufs, space="PSUM") as pp,
        ):
            iota_t = cp.tile([P, P], cmp_dt)
            nc.sync.dma_start(out=iota_t[:], in_=iota_i[:])

            for _rep in range(reps):
                ft2 = meta2 = None
                for b in range(nblk):
                    if pair_dma and b + 1 < nblk:
                        if b % 2 == 0:
                            ft2 = ftp2.tile([P, 2, k, HD], f32, tag="ft2")
                            nc.sync.dma_start(out=ft2[:], in_=ft_v2[b // 2])
                            meta2 = mp.tile([P, 2, k * H + k], f32, tag="m2")
                            small_dma.dma_start(
                                out=meta2[:], in_=s_v2[b // 2]
                            )
                        ft_blk = ft2[:, b % 2]
                        meta_blk = meta2[:, b % 2]
                    elif pair_dma:
                        # odd trailing block: single-block transfer
                        ft1 = ftp.tile([P, k, HD], f32, tag="ft1")
                        nc.sync.dma_start(out=ft1[:], in_=ft_v[b])
                        meta1 = mp.tile([P, k * H + k], f32, tag="m1")
                        small_dma.dma_start(out=meta1[:], in_=s_i[b])
                        ft_blk = ft1[:]
                        meta_blk = meta1[:]
                    else:
                        ft_blk_t = ftp.tile([P, k, HD], f32)
                        nc.sync.dma_start(out=ft_blk_t[:], in_=ft_v[b])
                        meta_blk_t = mp.tile([P, k * H + k], f32)
                        small_dma.dma_start(out=meta_blk_t[:], in_=s_i[b])
                        ft_blk = ft_blk_t[:]
                        meta_blk = meta_blk_t[:]
                    s_blk = meta_blk[:, : k * H]
                    d_blk = meta_blk[:, k * H :]

                    # vals layout [P, k, 260]: cols 0:256 = ex*ft, 256:260 = ex
                    vals_blk = vp.tile([P, k, HD + H], mm_dt)

                    # ex = exp(leaky_relu(s)); exp writes straight into the
                    # trailing 4 columns of each tile's vals slab
                    e_t = wp.tile([P, k * H], f32)
                    if act_lrelu:
                        nc.scalar.activation(
                            e_t[:], s_blk[:],
                            mybir.ActivationFunctionType.Lrelu, alpha=0.01,
                        )
                    else:
                        nc.vector.scalar_tensor_tensor(
                            out=e_t[:], in0=s_blk[:], scalar=0.01, in1=s_blk[:],
                            op0=mybir.AluOpType.mult, op1=mybir.AluOpType.max,
                        )
                    ex_t = wp.tile([P, k * H], f32, tag="ex_t")
                    nc.scalar.activation(
                        ex_t[:], e_t[:], mybir.ActivationFunctionType.Exp
                    )
                    ex_blk = vals_blk[:, :, HD : HD + H]
                    nc.vector.tensor_copy(
                        out=ex_blk, in_=ex_t[:].rearrange("p (t h) -> p t h", h=H)
                    )

                    # one-hot for all k tiles in one op:
                    # oh[e, t, n] = (iota[n] == dstl[e, t])
                    oh_blk = ohp.tile([P, k, P], mm_dt)
                    nc.vector.tensor_tensor(
                        out=oh_blk[:],
                        in0=iota_t[:, None, :].to_broadcast([P, k, P]),
                        in1=d_blk[:, :, None].to_broadcast([P, k, P]),
                        op=mybir.AluOpType.is_equal,
                    )

                    # vals[e, t, h, d] = ft[e, t, h, d] * ex[e, t, h]
                    kd = k - gp_tiles
                    nc.vector.tensor_tensor(
                        out=vals_blk[:, :kd, :HD].rearrange(
                            "p t (h d) -> p t h d", h=H
                        ),
                        in0=ft_blk[:, :kd].rearrange("p t (h d) -> p t h d", h=H),
                        in1=ex_blk[:, :kd, :, None].to_broadcast([P, kd, H, D]),
                        op=mybir.AluOpType.mult,
                    )
                    if gp_tiles:
                        nc.gpsimd.tensor_tensor(
                            out=vals_blk[:, kd:, :HD].rearrange(
                                "p t (h d) -> p t h d", h=H
                            ),
                            in0=ft_blk[:, kd:].rearrange(
                                "p t (h d) -> p t h d", h=H
                            ),
                            in1=ex_blk[:, kd:, :, None].to_broadcast(
                                [P, gp_tiles, H, D]
                            ),
                            op=mybir.AluOpType.mult,
                        )

                    # single matmul per tile accumulates num (0:256) + den
                    # (256:260) into one PSUM bank
                    acc = pp.tile([P, HD + H], f32, tag="acc")
                    for t in range(k):
                        nc.tensor.matmul(
                            acc[:], lhsT=oh_blk[:, t, :], rhs=vals_blk[:, t],
                            start=(t == 0), stop=(t == k - 1),
                        )

                    if gp_epi:
                        # ACT drains PSUM; GPSIMD does the broadcast multiply;
                        # DVE only does the small eps-add + reciprocal
                        acc_sb = op_.tile([P, HD + H], f32, tag="acc_sb")
                        nc.scalar.copy(acc_sb[:], acc[:])
                        den = wp.tile([P, H], f32, tag="den")
                        nc.vector.tensor_scalar_add(
                            den[:], acc_sb[:, HD : HD + H], 1e-30
                        )
                        rec = wp.tile([P, H], f32, tag="rec")
                        nc.vector.reciprocal(rec[:], den[:])
                        outsb = op_.tile([P, H, D], f32)
                        nc.gpsimd.tensor_tensor(
                            out=outsb[:],
                            in0=acc_sb[:, :HD].rearrange("p (h d) -> p h d", h=H),
                            in1=rec[:, :, None].to_broadcast([P, H, D]),
                            op=mybir.AluOpType.mult,
                        )
                    else:
                        den = wp.tile([P, H], f32, tag="den")
                        nc.vector.tensor_scalar_add(
                            den[:], acc[:, HD : HD + H], 1e-30
                        )
                        rec = wp.tile([P, H], f32, tag="rec")
                        nc.vector.reciprocal(rec[:], den[:])
                        outsb = op_.tile([P, H, D], f32)
                        nc.vector.tensor_tensor(
                            out=outsb[:],
                            in0=acc[:, :HD].rearrange("p (h d) -> p h d", h=H),
                            in1=rec[:, :, None].to_broadcast([P, H, D]),
                            op=mybir.AluOpType.mult,
                        )
                    small_dma.dma_start(
                        out=out_o[b * P : (b + 1) * P, :],
                        in_=outsb[:].rearrange("p h d -> p (h d)"),
                    )

    nc.compile()
    return nc


def kernel(a1, a2, ft, dst):
    global LAST_RESULT, LAST_NC, LAST_IN_MAPS
    a1 = np.asarray(a1, dtype=np.float32)
    a2 = np.asarray(a2, dtype=np.float32)
    ft = np.asarray(ft, dtype=np.float32)
    dst = np.asarray(dst)

    n = a1.shape[0]
    e = dst.shape[0]
    assert a1.shape == (n, H, 1) and a2.shape == (e, H, 1)
    assert ft.shape == (e, H, D)

    # ---- host prep: sort edges by dst, fold a1 gather ----
    order = np.argsort(dst, kind="stable")
    dst_s = dst[order].astype(np.int64)
    s_all = (a1[:, :, 0][dst_s] + a2[order, :, 0]).astype(np.float32)  # [E,H]
    ft_s = ft[order].reshape(e, HD)  # [E, 256]

    nblk_total = math.ceil(n / P)                      # 391
    nblk = math.ceil(nblk_total / N_CORES)             # 49 blocks per core
    npc = nblk * P                                     # 6272 nodes per core

    # edges per 128-node block (global)
    block_starts = np.searchsorted(dst_s, np.arange(0, (nblk * N_CORES) * P + 1, P))
    counts = np.diff(block_starts)                     # [nblk*8]
    k = max(1, int(math.ceil(counts.max() / P)))       # edge tiles per block
    epb = k * P                                        # padded edges per block

    # ---- pack per-core inputs ----
    iota_np = np.broadcast_to(
        np.arange(P, dtype=np.float32)[None, :], (P, P)
    ).copy()

    in_maps = []
    for c in range(N_CORES):
        ftp = np.zeros((nblk * epb, HD), dtype=np.float32)
        sp = np.full((nblk * epb, H), -1e9, dtype=np.float32)
        dp = np.zeros((nblk * epb,), dtype=np.float32)
        for bl in range(nblk):
            g = c * nblk + bl                          # global block id
            lo, hi = block_starts[g], block_starts[g + 1]
            cnt = hi - lo
            o = bl * epb
            ftp[o : o + cnt] = ft_s[lo:hi]
            sp[o : o + cnt] = s_all[lo:hi]
            dp[o : o + cnt] = (dst_s[lo:hi] - g * P).astype(np.float32)
        # swizzle ft to [nblk, P, k, HD] (contiguous per-partition runs)
        ft_sw = np.ascontiguousarray(
            ftp.reshape(nblk, k, P, HD).transpose(0, 2, 1, 3)
        )
        # swizzle: [nblk, k, P, x] -> [nblk, P, k, x]; meta = [s | dstl]
        s_sw = sp.reshape(nblk, k, P, H).transpose(0, 2, 1, 3).reshape(
            nblk, P, k * H
        )
        d_sw = dp.reshape(nblk, k, P).transpose(0, 2, 1)
        meta = np.ascontiguousarray(
            np.concatenate([s_sw, d_sw], axis=2, dtype=np.float32)
        )
        in_maps.append({"ft_i": ft_sw, "s_i": meta, "iota_i": iota_np})

    key = (nblk, k, OH_BF16, GP_TILES)
    if key not in _kernel_cache:
        _kernel_cache[key] = _build(
            nblk, k, bf16_oh=OH_BF16, gp_tiles=GP_TILES, dma_split=True,
            psum_bufs=6,
        )
    nc = _kernel_cache[key]

    try:
        res = run_bass_kernel_spmd(nc, in_maps, core_ids=list(range(N_CORES)))
    except Exception:
        # transient NRT_EXEC_UNIT_UNRECOVERABLE has been observed once on a
        # shared device; one retry clears it
        res = run_bass_kernel_spmd(nc, in_maps, core_ids=list(range(N_CORES)))
    LAST_RESULT = res
    LAST_NC = nc
    LAST_IN_MAPS = in_maps

    out = np.empty((n, H, D), dtype=np.float32)
    for c in range(N_CORES):
        lo = c * npc
        real = min(npc, n - lo)
        if real <= 0:
            break
        out[lo : lo + real] = res.results[c]["out_o"].reshape(npc, H, D)[:real]
    return out

